# revision 3
# baseline (speedup 1.0000x reference)
"""DTW (symmetric2, L1 cost) batch kernel for Trainium2, 8 NeuronCores.

Problem: 64 pairs of length-1024 fp32 sequences; per pair the full
1024x1024 DTW dynamic program; output = mean over pairs of
D[n-1, m-1] / (n + m).

Strategy per core (8 samples each):
  - Row-scan formulation: for each DP row,
        P[j]   = min(Dprev[j-1] + d[j], Dprev[j])
        D[j]   = min(P[j], D[j-1]) + d[j]
    The serial in-row recurrence maps onto the DVE tensor_tensor_scan
    instruction (op0=min, op1=add); the scan is widened to 65 columns
    with a BIG/0 leading element so out[0] passes the carry through,
    which doubles as next row's Dprev[-1] boundary (no copy needed).
  - Columns split into 16 chunks of 64; partition p = 8*chunk + sample.
    Chunks run in a software wavefront: chunk c processes row block
    i//R at macro-step tau = i//R + 2c (double skew gives the boundary
    transfer a full macro-step of slack).
  - The chunk->chunk boundary columns (a +8 partition shift, illegal
    for DVE access patterns) move through the TensorEngine once per
    macro-step: one [128x128]@[128x8] matmul with a shift matrix, plus
    a second accumulating matmul that writes BIG into chunk 0's rows.
  - Local cost rows d[j] = |x_i - y_j| are produced by the Scalar
    (activation) engine off the critical path.
  - Row state lives in a 16-slot rotating arena so the matmul reads
    all 8 right-boundary columns with one strided access pattern.
"""

import sys

sys.path.insert(0, "/opt/trn_rl_repo")

import numpy as np

import concourse.bass as bass
import concourse.bacc as bacc
import concourse.mybir as mybir
from concourse import tile
from concourse.bass_utils import run_bass_kernel_spmd

AF = mybir.ActivationFunctionType
ALU = mybir.AluOpType
FP32 = mybir.dt.float32

NCORES = 8
B = 8             # samples per core
N = 1024          # sequence length (rows == cols)
C = 16            # column chunks
W = N // C        # 64 columns per chunk
R = 8             # rows per macro-step
SKEW = 2          # macro-steps of lag between adjacent chunks
T = N // R + SKEW * (C - 1)   # 158 macro-steps
S_TOTAL = T * R               # 1264 row-steps
NSLOT = 16                    # row-state arena slots
BIG = 1.0e30

_CACHE = {}


def _build():
    nc = bacc.Bacc("TRN2", target_bir_lowering=False, debug=False)
    x8 = nc.declare_dram_parameter("x8", [B, N], FP32, isOutput=False)
    y8 = nc.declare_dram_parameter("y8", [B, N], FP32, isOutput=False)
    s8in = nc.declare_dram_parameter("s8", [128, 128], FP32, isOutput=False)
    out = nc.declare_dram_parameter("dists", [B, 1], FP32, isOutput=True)

    with tile.TileContext(nc) as tc:
        with (
            tc.tile_pool(name="persist", bufs=1) as pp,
            tc.tile_pool(name="qpool", bufs=2) as qpool,
            tc.tile_pool(name="psum", bufs=2, space=bass.MemorySpace.PSUM) as psp,
        ):
            S8 = pp.tile([128, 128], FP32, tag="s8t")
            W2 = pp.tile([128, 128], FP32, tag="w2t")
            ONES = pp.tile([128, R], FP32, tag="ones")
            Y = pp.tile([128, W], FP32, tag="y")
            XS = pp.tile([128, S_TOTAL], FP32, tag="xs")
            BB = pp.tile([128, NSLOT, W + 1], FP32, tag="bb")
            LF = [
                pp.tile([128, R + 1], FP32, name=f"lft{i}", tag=f"lf{i}")
                for i in range(3)
            ]
            ZC = pp.tile([128, 1], FP32, tag="zc")

            nc.sync.dma_start(S8[:], s8in[:])
            # X skew: XS[8c+b, s] = x[b, s - SKEW*R*c]; pad BIG so
            # out-of-range rows produce huge local costs (+inf rows).
            nc.vector.memset(XS[:], BIG)
            for c in range(C):
                o = SKEW * R * c
                nc.sync.dma_start(XS[8 * c : 8 * c + 8, o : o + N], x8[:])
                nc.sync.dma_start(Y[8 * c : 8 * c + 8, :], y8[:, W * c : W * c + W])
            # negate in place: bias for |y - x| activation
            nc.vector.tensor_scalar_mul(XS[:], XS[:], -1.0)

            nc.vector.memset(BB[:], BIG)
            for i in range(3):
                nc.vector.memset(LF[i][:], BIG)
            nc.vector.memset(ZC[:], BIG)
            nc.vector.memset(ZC[0:8, :], 0.0)
            # patch matmul: W2.T @ ONES adds BIG into partitions 0:8
            nc.vector.memset(W2[:], 0.0)
            nc.vector.memset(W2[0:1, 0:8], BIG)
            nc.vector.memset(ONES[:], 1.0)

            # d tiles [128, W+1]: col 0 stays 0 forever (scan leading
            # element); ACT writes cols 1..W each row. P tiles keep
            # col 0 = BIG forever.
            dts = [
                pp.tile([128, W + 1], FP32, name=f"dt{i}", tag=f"dt{i}")
                for i in range(4)
            ]
            pts = [
                pp.tile([128, W + 1], FP32, name=f"pt{i}", tag=f"pt{i}")
                for i in range(3)
            ]
            for t_ in dts:
                nc.vector.memset(t_[:, 0:1], 0.0)
            for t_ in pts:
                nc.vector.memset(t_[:, 0:1], BIG)

            pending = None  # deferred epilogue copies: (acc, lf_nxt)
            for tau in range(T):
                lf_cur = LF[tau % 3]
                lf_nxt = LF[(tau + 2) % 3]
                lf_mid = LF[(tau + 1) % 3]
                do_mm = tau < T - SKEW
                if do_mm:
                    acc = psp.tile([128, R], FP32, tag="acc", name="acc")
                    # boundary col for the block after next: available early
                    nc.vector.tensor_copy(lf_nxt[:, 0:1], lf_mid[:, R : R + 1])
                for r in range(R):
                    s = R * tau + r
                    b_prev = BB[:, (s - 1) % NSLOT, :]
                    b_cur = BB[:, s % NSLOT, :]
                    d = dts[s % 4]
                    nc.scalar.activation(
                        d[:, 1 : W + 1],
                        Y[:],
                        AF.Abs,
                        bias=XS[:, s : s + 1],
                        scale=1.0,
                    )
                    q = qpool.tile([128, W], FP32, tag="q", name="q")
                    nc.vector.tensor_tensor(
                        q[:], b_prev[:, 0:W], d[:, 1 : W + 1], op=ALU.add
                    )
                    p = pts[s % 3]
                    nc.vector.tensor_tensor(
                        p[:, 1 : W + 1], q[:], b_prev[:, 1 : W + 1], op=ALU.min
                    )
                    if s == 0:
                        # special: scan cols 1..W with zero-carry for chunk 0;
                        # boundary col written separately
                        nc.vector.tensor_tensor_scan(
                            b_cur[:, 1 : W + 1],
                            p[:, 1 : W + 1],
                            d[:, 1 : W + 1],
                            ZC[:, 0:1],
                            op0=ALU.min,
                            op1=ALU.add,
                        )
                        nc.vector.memset(b_cur[:, 0:1], BIG)
                    else:
                        # 65-wide scan: out[0] = carry (data0 BIG, data1 0),
                        # doubling as next row's Dprev[-1]
                        nc.vector.tensor_tensor_scan(
                            b_cur[:, 0 : W + 1],
                            p[:, 0 : W + 1],
                            d[:, 0 : W + 1],
                            lf_cur[:, r + 1 : r + 2],
                            op0=ALU.min,
                            op1=ALU.add,
                        )
                    if r == 2 and pending is not None:
                        # deferred from previous macro-step: by now the PE
                        # matmuls have long finished, so no DVE stall
                        p_acc, p_lf = pending
                        nc.vector.tensor_copy(p_lf[:, 1 : R + 1], p_acc[:, 0:R])
                        pending = None
                if do_mm:
                    k0 = (R * tau) % NSLOT
                    nc.tensor.matmul(
                        acc[:, 0:R],
                        S8[:],
                        BB[:, k0 : k0 + R, W],
                        start=True,
                        stop=False,
                    )
                    nc.tensor.matmul(
                        acc[:, 0:R],
                        W2[:],
                        ONES[:],
                        start=False,
                        stop=True,
                        skip_group_check=True,
                    )
                    pending = (acc, lf_nxt)
            if pending is not None:
                p_acc, p_lf = pending
                nc.vector.tensor_copy(p_lf[:, 1 : R + 1], p_acc[:, 0:R])

            last_cur = BB[:, (S_TOTAL - 1) % NSLOT, :]
            nc.sync.dma_start(out[:], last_cur[120:128, W : W + 1])

    nc.compile()
    return nc


def _shift_matrix():
    s8 = np.zeros((128, 128), np.float32)
    for r in range(120):
        s8[r, r + 8] = 1.0  # out[p] = in[p - 8]
    return s8


LAST = {}


def kernel(x: np.ndarray, x_target: np.ndarray) -> np.ndarray:
    import os

    x = np.ascontiguousarray(np.asarray(x, np.float32))
    y = np.ascontiguousarray(np.asarray(x_target, np.float32))
    if "nc" not in _CACHE:
        _CACHE["nc"] = _build()
    nc = _CACHE["nc"]
    s8 = _shift_matrix()
    in_maps = [
        {"x8": x[8 * k : 8 * k + 8], "y8": y[8 * k : 8 * k + 8], "s8": s8}
        for k in range(NCORES)
    ]
    trace = bool(os.environ.get("DTW_TRACE"))
    r = run_bass_kernel_spmd(nc, in_maps, list(range(NCORES)), trace=trace)
    LAST["exec_time_ns"] = r.exec_time_ns
    LAST["profile_json"] = r.profile_json
    LAST["trace_path"] = (
        r.instructions_and_trace[1] if r.instructions_and_trace else None
    )
    res = r.results
    dists = np.concatenate([r["dists"][:, 0] for r in res]).astype(np.float32)
    dists = dists / np.float32(2.0 * N)
    return np.float32(np.mean(dists))



# revision 4
# speedup vs baseline: 2.0609x; 2.0609x over previous
"""DTW (symmetric2, L1 cost) batch kernel for Trainium2, 8 NeuronCores.

Problem: 64 pairs of length-1024 fp32 sequences; per pair the full
1024x1024 DTW dynamic program; output = mean over pairs of
D[n-1, m-1] / (n + m).

Strategy per core (8 samples each):
  - Row-scan formulation: for each DP row,
        q[j]   = Dprev[j-1] + d[j]
        p[j]   = min(q[j], Dprev[j])
        D[j]   = min(p[j], D[j-1]) + d[j]
    The serial in-row recurrence maps onto the DVE tensor_tensor_scan
    instruction (op0=min, op1=add); the scan is widened to 65 columns
    with a BIG/0 leading element so out[0] passes the carry through,
    which doubles as next row's Dprev[-1] boundary (no copy needed).
  - Columns split into 16 chunks of 64; partition p = 8*chunk + sample.
    Chunks run in a software wavefront: R=4 rows per macro-step, each
    chunk lagging its left neighbour by SKEW=2 macro-steps (8 rows).
  - The chunk->chunk boundary columns (a +8 partition shift) move via
    one SBUF->SBUF DMA per macro-step on the otherwise-idle Sync
    queue, writing partitions 8:128 of a rotating lf tile; partitions
    0:8 (chunk 0) stay BIG from a single init memset.
  - Local cost rows d[j] = |x_i - y_j| are produced by the Scalar
    (activation) engine one macro-step ahead, emitted in reverse row
    order per macro-step so the Tile framework's dominated-wait
    elision collapses the DVE-side ACT waits to one per macro-step.
  - Row state lives in a 16-slot rotating arena so the boundary DMA
    reads all R right-boundary columns with one strided access
    pattern.
"""

import sys

sys.path.insert(0, "/opt/trn_rl_repo")

import numpy as np

import concourse.bass as bass
import concourse.bacc as bacc
import concourse.mybir as mybir
from concourse import tile
from concourse.bass_utils import run_bass_kernel_spmd

AF = mybir.ActivationFunctionType
ALU = mybir.AluOpType
FP32 = mybir.dt.float32

NCORES = 8
B = 8             # samples per core
N = 1024          # sequence length (rows == cols)
C = 16            # column chunks
W = N // C        # 64 columns per chunk
R = 4             # rows per macro-step
SKEW = 2          # macro-steps of lag between adjacent chunks
T = N // R + SKEW * (C - 1)   # 286 macro-steps
S_TOTAL = T * R               # 1144 row-steps
NSLOT = 16                    # row-state arena slots
ND = 2 * R                    # d-tile ring: two macro-steps in flight
BIG = 1.0e30

_CACHE = {}


def _build():
    nc = bacc.Bacc("TRN2", target_bir_lowering=False, debug=False)
    x8 = nc.declare_dram_parameter("x8", [B, N], FP32, isOutput=False)
    y8 = nc.declare_dram_parameter("y8", [B, N], FP32, isOutput=False)
    out = nc.declare_dram_parameter("dists", [B, 1], FP32, isOutput=True)

    with tile.TileContext(nc) as tc:
        with (
            tc.tile_pool(name="persist", bufs=1) as pp,
            tc.tile_pool(name="qpool", bufs=2) as qpool,
        ):
            Y = pp.tile([128, W], FP32, tag="y")
            XS = pp.tile([128, S_TOTAL], FP32, tag="xs")
            BB = pp.tile([128, NSLOT, W + 1], FP32, tag="bb")
            LF = [
                pp.tile([128, R + 1], FP32, name=f"lft{i}", tag=f"lf{i}")
                for i in range(3)
            ]
            ZC = pp.tile([128, 1], FP32, tag="zc")

            # X skew: XS[8c+b, s] = x[b, s - SKEW*R*c]; pad BIG so
            # out-of-range rows produce huge local costs (+inf rows).
            nc.vector.memset(XS[:], BIG)
            for c in range(C):
                o = SKEW * R * c
                nc.sync.dma_start(XS[8 * c : 8 * c + 8, o : o + N], x8[:])
                nc.sync.dma_start(Y[8 * c : 8 * c + 8, :], y8[:, W * c : W * c + W])
            # negate in place: bias for |y - x| activation
            nc.vector.tensor_scalar_mul(XS[:], XS[:], -1.0)

            nc.vector.memset(BB[:], BIG)
            for i in range(3):
                nc.vector.memset(LF[i][:], BIG)
            nc.vector.memset(ZC[:], BIG)
            nc.vector.memset(ZC[0:8, :], 0.0)

            # d tiles [128, W+1]: col 0 stays 0 forever (scan leading
            # element); ACT writes cols 1..W each row. P tiles keep
            # col 0 = BIG forever.
            dts = [
                pp.tile([128, W + 1], FP32, name=f"dt{i}", tag=f"dt{i}")
                for i in range(ND)
            ]
            pts = [
                pp.tile([128, W + 1], FP32, name=f"pt{i}", tag=f"pt{i}")
                for i in range(3)
            ]
            for t_ in dts:
                nc.vector.memset(t_[:, 0:1], 0.0)
            for t_ in pts:
                nc.vector.memset(t_[:, 0:1], BIG)

            def act_batch(tau):
                # d rows for macro-step tau, reverse order: the first
                # row's wait dominates the rest on the consumer side.
                for r in range(R - 1, -1, -1):
                    s = R * tau + r
                    nc.scalar.activation(
                        dts[s % ND][:, 1 : W + 1],
                        Y[:],
                        AF.Abs,
                        bias=XS[:, s : s + 1],
                        scale=1.0,
                    )

            act_batch(0)
            act_batch(1)
            for tau in range(T):
                if tau + 2 < T:
                    act_batch(tau + 2)
                lf_cur = LF[tau % 3]
                lf_nxt = LF[(tau + 2) % 3]
                for r in range(R):
                    s = R * tau + r
                    b_prev = BB[:, (s - 1) % NSLOT, :]
                    b_cur = BB[:, s % NSLOT, :]
                    d = dts[s % ND]
                    q = qpool.tile([128, W], FP32, tag="q", name="q")
                    nc.vector.tensor_tensor(
                        q[:], b_prev[:, 0:W], d[:, 1 : W + 1], op=ALU.add
                    )
                    p = pts[s % 3]
                    nc.vector.tensor_tensor(
                        p[:, 1 : W + 1], q[:], b_prev[:, 1 : W + 1], op=ALU.min
                    )
                    if s == 0:
                        # special: scan cols 1..W with zero-carry for chunk 0;
                        # boundary col written separately
                        nc.vector.tensor_tensor_scan(
                            b_cur[:, 1 : W + 1],
                            p[:, 1 : W + 1],
                            d[:, 1 : W + 1],
                            ZC[:, 0:1],
                            op0=ALU.min,
                            op1=ALU.add,
                        )
                        nc.vector.memset(b_cur[:, 0:1], BIG)
                    else:
                        # 65-wide scan: out[0] = carry (data0 BIG, data1 0),
                        # doubling as next row's Dprev[-1]
                        nc.vector.tensor_tensor_scan(
                            b_cur[:, 0 : W + 1],
                            p[:, 0 : W + 1],
                            d[:, 0 : W + 1],
                            lf_cur[:, r + 1 : r + 2],
                            op0=ALU.min,
                            op1=ALU.add,
                        )
                if tau < T - SKEW:
                    # boundary transfer: right-boundary cols of the R rows
                    # just written, shifted +8 partitions, into the lf tile
                    # that macro-step tau+2 will read.
                    k0 = (R * tau) % NSLOT
                    nc.sync.dma_start(
                        lf_nxt[8:128, 1 : R + 1],
                        BB[0:120, k0 : k0 + R, W : W + 1],
                    )

            last_cur = BB[:, (S_TOTAL - 1) % NSLOT, :]
            nc.sync.dma_start(out[:], last_cur[120:128, W : W + 1])

    nc.compile()
    return nc


LAST = {}


def kernel(x: np.ndarray, x_target: np.ndarray) -> np.ndarray:
    import os

    x = np.ascontiguousarray(np.asarray(x, np.float32))
    y = np.ascontiguousarray(np.asarray(x_target, np.float32))
    if "nc" not in _CACHE:
        _CACHE["nc"] = _build()
    nc = _CACHE["nc"]
    in_maps = [
        {"x8": x[8 * k : 8 * k + 8], "y8": y[8 * k : 8 * k + 8]}
        for k in range(NCORES)
    ]
    trace = bool(os.environ.get("DTW_TRACE"))
    r = run_bass_kernel_spmd(nc, in_maps, list(range(NCORES)), trace=trace)
    LAST["exec_time_ns"] = r.exec_time_ns
    LAST["profile_json"] = r.profile_json
    LAST["trace_path"] = (
        r.instructions_and_trace[1] if r.instructions_and_trace else None
    )
    res = r.results
    dists = np.concatenate([r["dists"][:, 0] for r in res]).astype(np.float32)
    dists = dists / np.float32(2.0 * N)
    return np.float32(np.mean(dists))


# revision 5
# speedup vs baseline: 2.0834x; 1.0109x over previous
"""DTW (symmetric2, L1 cost) batch kernel for Trainium2, 8 NeuronCores.

Problem: 64 pairs of length-1024 fp32 sequences; per pair the full
1024x1024 DTW dynamic program; output = mean over pairs of
D[n-1, m-1] / (n + m).

Raw-bass hand-scheduled implementation (no Tile framework):
  - Row-scan DP per core (8 samples, partition p = 8*chunk + sample):
        q[j] = Dprev[j-1] + d[j];  p[j] = min(q[j], Dprev[j])
        D[j] = min(p[j], D[j-1]) + d[j]   (DVE tensor_tensor_scan)
  - 16 column chunks of 64 in a software wavefront, R=4 rows per
    macro-step, SKEW=3 macro-steps of lag between adjacent chunks.
  - Same-engine RAW hazards are left to the DVE's in-order pipeline +
    drain (no per-op semaphores), so the q/p/scan chain streams at
    ~0.55 us per DP row-step; cross-engine sync happens once per
    macro-step.
  - Chunk boundary columns move via one SBUF->SBUF partition-shifted
    DMA per macro-step on the Sync queue. The DMA's semaphore posts
    16 progress bumps spread across the transfer (not a single
    completion bump), so the consumer waits one extra whole DMA
    (SM >= 16*(tau-SKEW+2)) for write-visibility settle; determinism
    verified against this (bit-exact across repeated runs).
  - Local-cost rows d = |y - x_i| come from the Scalar engine
    (activation, scale=-1, bias=x_i), one macro-step batch ahead.
"""

import sys

sys.path.insert(0, "/opt/trn_rl_repo")

import numpy as np

import concourse.bass as bass
import concourse.mybir as mybir
from concourse.bass_utils import run_bass_kernel_spmd

AF = mybir.ActivationFunctionType
ALU = mybir.AluOpType
FP32 = mybir.dt.float32

NCORES = 8
B = 8             # samples per core
N = 1024          # sequence length
C = 16            # column chunks
W = N // C        # 64
R = 4             # rows per macro-step
SKEW = 3          # macro-steps of lag between adjacent chunks
T = N // R + SKEW * (C - 1)   # 301
S_TOTAL = T * R               # 1204
NSLOT = 16
ND = 12           # d-row ring: 3 macro-step batches in flight
NLF = 4           # lf ring
BIG = 1.0e30

_CACHE = {}


def _build():
    nc = bass.Bass("TRN2", target_bir_lowering=False)

    x8 = nc.dram_tensor("x8", [B, N], FP32, kind="ExternalInput")
    y8 = nc.dram_tensor("y8", [B, N], FP32, kind="ExternalInput")
    out = nc.dram_tensor("dists", [B, 1], FP32, kind="ExternalOutput")

    with (
        nc.Block() as block,
        nc.semaphore("dma_in") as dma_in,
        nc.semaphore("sa") as SA,
        nc.semaphore("sd") as SD,
        nc.semaphore("sm") as SM,
        nc.semaphore("v_memset") as v_memset,
        nc.semaphore("dma_out") as dma_out,
        # hot steady-state tensors first: their SBUF offsets match the
        # layout that measured 515 ns/row-step
        nc.sbuf_tensor("Y", [128, W], FP32) as Y,
        nc.sbuf_tensor("BB", [128, NSLOT, W + 1], FP32) as BB,
        nc.sbuf_tensor("LF", [128, NLF, R + 1], FP32) as LF,
        nc.sbuf_tensor("DTS", [128, ND, W + 1], FP32) as DTS,
        nc.sbuf_tensor("PTS", [128, 3, W + 1], FP32) as PTS,
        nc.sbuf_tensor("QQ", [128, 2, W], FP32) as QQ,
        nc.sbuf_tensor("ZC", [128, 1], FP32) as ZC,
        nc.sbuf_tensor("XS", [128, S_TOTAL], FP32) as XS,
    ):
        N_IN_DMAS = C + 1  # 16 XS slices + 1 combined Y

        @block.sync
        def _(sync):
            sync.wait_ge(v_memset, 1)
            for c in range(C):
                o = SKEW * R * c
                sync.dma_start(XS[8 * c : 8 * c + 8, o : o + N], x8[:, :]).then_inc(
                    dma_in, 16
                )
            # Y[8c+b, j] = y[b, 64c+j]: one DMA, src iterates (c, b, j)
            sync.dma_start(
                Y[:, :], bass.AP(y8, 0, [[W, C], [N, B], [1, W]])
            ).then_inc(dma_in, 16)
            with nc.allow_non_contiguous_dma(reason="R-elem boundary gather"):
                for tau in range(T - SKEW):
                    sync.wait_ge(SD, tau + 1)
                    k0 = (R * tau) % NSLOT
                    sync.dma_start(
                        LF[8:128, (tau + SKEW) % NLF, 1 : R + 1],
                        BB[0:120, k0 : k0 + R, W : W + 1],
                    ).then_inc(SM, 16)
                sync.wait_ge(SD, T + 1)
                for _ in range(10):
                    sync.wait_ge(SD, T + 1)  # settle spin ~0.5us
                sync.dma_start(
                    out[:, :], BB[120:128, (S_TOTAL - 1) % NSLOT, W : W + 1]
                ).then_inc(dma_out, 16)
            sync.wait_ge(dma_out, 16)

        @block.scalar
        def _(scalar):
            scalar.wait_ge(v_memset, 1)
            scalar.wait_ge(dma_in, 16 * N_IN_DMAS)

            def act_batch(tau):
                for r in range(R):
                    s = R * tau + r
                    ins = nc.scalar.activation(
                        DTS[:, s % ND, 1 : W + 1],
                        Y[:, :],
                        AF.Abs,
                        bias=XS[:, s : s + 1],
                        scale=-1.0,
                    )
                    if r == R - 1:
                        ins.then_inc(SA, 1)

            act_batch(0)
            act_batch(1)
            act_batch(2)
            for tau in range(3, T):
                scalar.wait_ge(SD, tau - 2)
                act_batch(tau)

        @block.vector
        def _(vector):
            nc.vector.memset(XS[:, :], BIG)
            nc.vector.memset(BB[:, :, :], BIG)
            nc.vector.memset(LF[:, :, :], BIG)
            nc.vector.memset(ZC[:, :], BIG)
            nc.vector.memset(ZC[0:8, :], 0.0)
            nc.vector.memset(DTS[:, :, 0:1], 0.0)
            nc.vector.memset(PTS[:, :, 0:1], BIG).then_inc(v_memset, 1)

            for tau in range(T):
                vector.wait_ge(SA, tau + 1)
                lf = LF[:, tau % NLF, :]
                for r in range(R):
                    s = R * tau + r
                    b_prev = BB[:, (s - 1) % NSLOT, :]
                    b_cur = BB[:, s % NSLOT, :]
                    d = DTS[:, s % ND, :]
                    q = QQ[:, s % 2, :]
                    p = PTS[:, s % 3, :]
                    nc.vector.tensor_tensor(
                        q[:, :], b_prev[:, 0:W], d[:, 1 : W + 1], op=ALU.add
                    )
                    nc.vector.tensor_tensor(
                        p[:, 1 : W + 1], q[:, :], b_prev[:, 1 : W + 1], op=ALU.min
                    )
                    if r == 0 and tau >= SKEW:
                        vector.wait_ge(SM, 16 * min(tau - SKEW + 2, T - SKEW))
                    if s == 0:
                        nc.vector.tensor_tensor_scan(
                            b_cur[:, 1 : W + 1],
                            p[:, 1 : W + 1],
                            d[:, 1 : W + 1],
                            ZC[:, 0:1],
                            op0=ALU.min,
                            op1=ALU.add,
                        )
                        ins = nc.vector.memset(b_cur[:, 0:1], BIG)
                    else:
                        ins = nc.vector.tensor_tensor_scan(
                            b_cur[:, 0 : W + 1],
                            p[:, 0 : W + 1],
                            d[:, 0 : W + 1],
                            lf[:, r + 1 : r + 2],
                            op0=ALU.min,
                            op1=ALU.add,
                        )
                    if r == R - 1:
                        ins.then_inc(SD, 1)

            # drain padding after the final scan: give its SBUF writes
            # time to land before the output DMA reads them
            nc.vector.memset(QQ[:, 0, :], 0.0)
            nc.vector.memset(QQ[:, 1, :], 0.0)
            nc.vector.memset(QQ[:, 0, :], 0.0).then_inc(SD, 1)

    return nc


LAST = {}


def kernel(x: np.ndarray, x_target: np.ndarray) -> np.ndarray:
    import os

    x = np.ascontiguousarray(np.asarray(x, np.float32))
    y = np.ascontiguousarray(np.asarray(x_target, np.float32))
    if "nc" not in _CACHE:
        _CACHE["nc"] = _build()
    nc = _CACHE["nc"]
    in_maps = [
        {"x8": x[8 * k : 8 * k + 8], "y8": y[8 * k : 8 * k + 8]}
        for k in range(NCORES)
    ]
    trace = bool(os.environ.get("DTW_TRACE"))
    r = run_bass_kernel_spmd(nc, in_maps, list(range(NCORES)), trace=trace)
    LAST["exec_time_ns"] = r.exec_time_ns
    LAST["profile_json"] = r.profile_json
    LAST["trace_path"] = (
        r.instructions_and_trace[1] if r.instructions_and_trace else None
    )
    res = r.results
    dists = np.concatenate([rr["dists"][:, 0] for rr in res]).astype(np.float32)
    dists = dists / np.float32(2.0 * N)
    return np.float32(np.mean(dists))


# revision 6
# speedup vs baseline: 2.0851x; 1.0008x over previous
"""DTW (symmetric2, L1 cost) batch kernel for Trainium2, 8 NeuronCores.

Problem: 64 pairs of length-1024 fp32 sequences; per pair the full
1024x1024 DTW dynamic program; output = mean over pairs of
D[n-1, m-1] / (n + m).

Raw-bass hand-scheduled implementation (no Tile framework):
  - Row-scan DP per core (8 samples, partition p = 8*chunk + sample):
        q[j] = Dprev[j-1] + d[j];  p[j] = min(q[j], Dprev[j])
        D[j] = min(p[j], D[j-1]) + d[j]   (DVE tensor_tensor_scan)
  - 16 column chunks of 64 in a software wavefront, R=4 rows per
    macro-step, SKEW=3 macro-steps of lag between adjacent chunks.
  - Same-engine RAW hazards are left to the DVE's in-order pipeline +
    drain (no per-op semaphores), so the q/p/scan chain streams at
    ~0.55 us per DP row-step; cross-engine sync happens once per
    macro-step.
  - Chunk boundary columns move via one SBUF->SBUF partition-shifted
    DMA per macro-step on the Sync queue. The DMA's semaphore posts
    16 progress bumps spread across the transfer (not a single
    completion bump), so the consumer waits one extra whole DMA
    (SM >= 16*(tau-SKEW+2)) for write-visibility settle; determinism
    verified against this (bit-exact across repeated runs).
  - Local-cost rows d = |y - x_i| come from the Scalar engine
    (activation, scale=-1, bias=x_i), one macro-step batch ahead.
"""

import sys

sys.path.insert(0, "/opt/trn_rl_repo")

import numpy as np

import concourse.bass as bass
import concourse.mybir as mybir
from concourse.bass_utils import run_bass_kernel_spmd

AF = mybir.ActivationFunctionType
ALU = mybir.AluOpType
FP32 = mybir.dt.float32

NCORES = 8
B = 8             # samples per core
N = 1024          # sequence length
C = 16            # column chunks
W = N // C        # 64
R = 4             # rows per macro-step
SKEW = 3          # macro-steps of lag between adjacent chunks
T = N // R + SKEW * (C - 1)   # 301
S_TOTAL = T * R               # 1204
NSLOT = 16
ND = 12           # d-row ring: 3 macro-step batches in flight
NLF = 4           # lf ring
BIG = 1.0e30

_CACHE = {}


def _build():
    nc = bass.Bass("TRN2", target_bir_lowering=False)

    x8 = nc.dram_tensor("x8", [B, N], FP32, kind="ExternalInput")
    y8 = nc.dram_tensor("y8", [B, N], FP32, kind="ExternalInput")
    out = nc.dram_tensor("dists", [B, 1], FP32, kind="ExternalOutput")

    with (
        nc.Block() as block,
        nc.semaphore("dma_in") as dma_in,
        nc.semaphore("sa") as SA,
        nc.semaphore("sd") as SD,
        nc.semaphore("sm") as SM,
        nc.semaphore("v_memset") as v_memset,
        nc.semaphore("dma_out") as dma_out,
        # hot steady-state tensors first: their SBUF offsets match the
        # layout that measured 515 ns/row-step
        nc.sbuf_tensor("Y", [128, W], FP32) as Y,
        nc.sbuf_tensor("BB", [128, NSLOT, W + 1], FP32) as BB,
        nc.sbuf_tensor("LF", [128, NLF, R + 1], FP32) as LF,
        nc.sbuf_tensor("DTS", [128, ND, W + 1], FP32) as DTS,
        nc.sbuf_tensor("PTS", [128, 3, W + 1], FP32) as PTS,
        nc.sbuf_tensor("QQ", [128, 2, W], FP32) as QQ,
        nc.sbuf_tensor("ZC", [128, 1], FP32) as ZC,
        nc.sbuf_tensor("XS", [128, S_TOTAL], FP32) as XS,
    ):
        N_IN_DMAS = C + 1  # 16 XS slices + 1 combined Y

        @block.sync
        def _(sync):
            sync.wait_ge(v_memset, 1)
            # Y[8c+b, j] = y[b, 64c+j]: one DMA, src iterates (c, b, j)
            sync.dma_start(
                Y[:, :], bass.AP(y8, 0, [[W, C], [N, B], [1, W]])
            ).then_inc(dma_in, 16)
            for c in range(C):
                o = SKEW * R * c
                sync.dma_start(XS[8 * c : 8 * c + 8, o : o + N], x8[:, :]).then_inc(
                    dma_in, 16
                )
            with nc.allow_non_contiguous_dma(reason="R-elem boundary gather"):
                for tau in range(T - SKEW):
                    sync.wait_ge(SD, tau + 1)
                    k0 = (R * tau) % NSLOT
                    sync.dma_start(
                        LF[8:128, (tau + SKEW) % NLF, 1 : R + 1],
                        BB[0:120, k0 : k0 + R, W : W + 1],
                    ).then_inc(SM, 16)
                sync.wait_ge(SD, T + 1)
                for _ in range(10):
                    sync.wait_ge(SD, T + 1)  # settle spin ~0.5us
                sync.dma_start(
                    out[:, :], BB[120:128, (S_TOTAL - 1) % NSLOT, W : W + 1]
                ).then_inc(dma_out, 16)
            sync.wait_ge(dma_out, 16)

        @block.scalar
        def _(scalar):
            scalar.wait_ge(v_memset, 1)

            def dma_gate(tau):
                need = min(2 + (R * tau + R - 1) // (SKEW * R), N_IN_DMAS)
                scalar.wait_ge(dma_in, 16 * need)

            def act_batch(tau):
                for r in range(R):
                    s = R * tau + r
                    ins = nc.scalar.activation(
                        DTS[:, s % ND, 1 : W + 1],
                        Y[:, :],
                        AF.Abs,
                        bias=XS[:, s : s + 1],
                        scale=-1.0,
                    )
                    if r == R - 1:
                        ins.then_inc(SA, 1)

            for tau in range(3):
                dma_gate(tau)
                act_batch(tau)
            for tau in range(3, T):
                dma_gate(tau)
                scalar.wait_ge(SD, tau - 2)
                act_batch(tau)

        @block.vector
        def _(vector):
            nc.vector.memset(XS[:, :], BIG)
            nc.vector.memset(BB[:, :, :], BIG)
            nc.vector.memset(LF[:, :, :], BIG)
            nc.vector.memset(ZC[:, :], BIG)
            nc.vector.memset(ZC[0:8, :], 0.0)
            nc.vector.memset(DTS[:, :, 0:1], 0.0)
            nc.vector.memset(PTS[:, :, 0:1], BIG).then_inc(v_memset, 1)

            for tau in range(T):
                vector.wait_ge(SA, tau + 1)
                lf = LF[:, tau % NLF, :]
                for r in range(R):
                    s = R * tau + r
                    b_prev = BB[:, (s - 1) % NSLOT, :]
                    b_cur = BB[:, s % NSLOT, :]
                    d = DTS[:, s % ND, :]
                    q = QQ[:, s % 2, :]
                    p = PTS[:, s % 3, :]
                    nc.vector.tensor_tensor(
                        q[:, :], b_prev[:, 0:W], d[:, 1 : W + 1], op=ALU.add
                    )
                    nc.vector.tensor_tensor(
                        p[:, 1 : W + 1], q[:, :], b_prev[:, 1 : W + 1], op=ALU.min
                    )
                    if r == 0 and tau >= SKEW:
                        vector.wait_ge(SM, 16 * min(tau - SKEW + 2, T - SKEW))
                    if s == 0:
                        nc.vector.tensor_tensor_scan(
                            b_cur[:, 1 : W + 1],
                            p[:, 1 : W + 1],
                            d[:, 1 : W + 1],
                            ZC[:, 0:1],
                            op0=ALU.min,
                            op1=ALU.add,
                        )
                        ins = nc.vector.memset(b_cur[:, 0:1], BIG)
                    else:
                        ins = nc.vector.tensor_tensor_scan(
                            b_cur[:, 0 : W + 1],
                            p[:, 0 : W + 1],
                            d[:, 0 : W + 1],
                            lf[:, r + 1 : r + 2],
                            op0=ALU.min,
                            op1=ALU.add,
                        )
                    if r == R - 1:
                        ins.then_inc(SD, 1)

            # drain padding after the final scan: give its SBUF writes
            # time to land before the output DMA reads them
            nc.vector.memset(QQ[:, 0, :], 0.0)
            nc.vector.memset(QQ[:, 1, :], 0.0)
            nc.vector.memset(QQ[:, 0, :], 0.0).then_inc(SD, 1)

    return nc


LAST = {}


def kernel(x: np.ndarray, x_target: np.ndarray) -> np.ndarray:
    import os

    x = np.ascontiguousarray(np.asarray(x, np.float32))
    y = np.ascontiguousarray(np.asarray(x_target, np.float32))
    if "nc" not in _CACHE:
        _CACHE["nc"] = _build()
    nc = _CACHE["nc"]
    in_maps = [
        {"x8": x[8 * k : 8 * k + 8], "y8": y[8 * k : 8 * k + 8]}
        for k in range(NCORES)
    ]
    trace = bool(os.environ.get("DTW_TRACE"))
    r = run_bass_kernel_spmd(nc, in_maps, list(range(NCORES)), trace=trace)
    LAST["exec_time_ns"] = r.exec_time_ns
    LAST["profile_json"] = r.profile_json
    LAST["trace_path"] = (
        r.instructions_and_trace[1] if r.instructions_and_trace else None
    )
    res = r.results
    dists = np.concatenate([rr["dists"][:, 0] for rr in res]).astype(np.float32)
    dists = dists / np.float32(2.0 * N)
    return np.float32(np.mean(dists))


# revision 7
# speedup vs baseline: 2.1241x; 1.0187x over previous
"""DTW (symmetric2, L1 cost) batch kernel for Trainium2, 8 NeuronCores.

Problem: 64 pairs of length-1024 fp32 sequences; per pair the full
1024x1024 DTW dynamic program; output = mean over pairs of
D[n-1, m-1] / (n + m).

Raw-bass hand-scheduled implementation (no Tile framework):
  - Row-scan DP per core (8 samples, partition p = 8*chunk + sample):
        q[j] = Dprev[j-1] + d[j];  p[j] = min(q[j], Dprev[j])
        D[j] = min(p[j], D[j-1]) + d[j]   (DVE tensor_tensor_scan)
  - 16 column chunks of 64 in a software wavefront, R=4 rows per
    macro-step, SKEW=3 macro-steps of lag between adjacent chunks.
  - Same-engine RAW hazards are left to the DVE's in-order pipeline +
    drain (no per-op semaphores), so the q/p/scan chain streams at
    ~0.55 us per DP row-step; cross-engine sync happens once per
    macro-step.
  - Chunk boundary columns move via one SBUF->SBUF partition-shifted
    DMA per macro-step on the Sync queue. The DMA's semaphore posts
    16 progress bumps spread across the transfer (not a single
    completion bump), so the consumer waits one extra whole DMA
    (SM >= 16*(tau-SKEW+2)) for write-visibility settle; determinism
    verified against this (bit-exact across repeated runs).
  - Local-cost rows d = |y - x_i| come from the Scalar engine
    (activation, scale=-1, bias=x_i), one macro-step batch ahead.
"""

import sys

sys.path.insert(0, "/opt/trn_rl_repo")

import numpy as np

import concourse.bass as bass
import concourse.mybir as mybir
from concourse.bass_utils import run_bass_kernel_spmd

AF = mybir.ActivationFunctionType
ALU = mybir.AluOpType
FP32 = mybir.dt.float32

NCORES = 8
B = 8             # samples per core
N = 1024          # sequence length
C = 16            # column chunks
W = N // C        # 64
R = 4             # rows per macro-step
SKEW = 3          # macro-steps of lag between adjacent chunks
T = N // R + SKEW * (C - 1)   # 301
S_TOTAL = T * R               # 1204
NSLOT = 16
ND = 12           # d-row ring: 3 macro-step batches in flight
NLF = 4           # lf ring
BIG = 1.0e30

_CACHE = {}


def _build():
    nc = bass.Bass("TRN2", target_bir_lowering=False)

    x8 = nc.dram_tensor("x8", [B, N], FP32, kind="ExternalInput")
    y8 = nc.dram_tensor("y8", [B, N], FP32, kind="ExternalInput")
    out = nc.dram_tensor("dists", [B, 1], FP32, kind="ExternalOutput")

    with (
        nc.Block() as block,
        nc.semaphore("dma_in") as dma_in,
        nc.semaphore("sa") as SA,
        nc.semaphore("sd") as SD,
        nc.semaphore("sm") as SM,
        nc.semaphore("v_memset") as v_memset,
        nc.semaphore("dma_out") as dma_out,
        # hot steady-state tensors first: their SBUF offsets match the
        # layout that measured 515 ns/row-step
        nc.sbuf_tensor("Y", [128, W], FP32) as Y,
        nc.sbuf_tensor("BB", [128, NSLOT, W + 1], FP32) as BB,
        nc.sbuf_tensor("LF", [128, NLF, R + 1], FP32) as LF,
        nc.sbuf_tensor("DTS", [128, ND, W + 1], FP32) as DTS,
        nc.sbuf_tensor("PTS", [128, 3, W + 1], FP32) as PTS,
        nc.sbuf_tensor("QQ", [128, 2, W], FP32) as QQ,
        nc.sbuf_tensor("ZC", [128, 1], FP32) as ZC,
        nc.sbuf_tensor("XS", [128, S_TOTAL], FP32) as XS,
    ):
        N_IN_DMAS = C + 1  # 16 XS slices + 1 combined Y

        @block.sync
        def _(sync):
            sync.wait_ge(v_memset, 1)
            # Y[8c+b, j] = y[b, 64c+j]: one DMA, src iterates (c, b, j)
            sync.dma_start(
                Y[:, :], bass.AP(y8, 0, [[W, C], [N, B], [1, W]])
            ).then_inc(dma_in, 16)
            for c in range(C):
                o = SKEW * R * c
                sync.dma_start(XS[8 * c : 8 * c + 8, o : o + N], x8[:, :]).then_inc(
                    dma_in, 16
                )
            with nc.allow_non_contiguous_dma(reason="R-elem boundary gather"):
                for tau in range(T - SKEW):
                    sync.wait_ge(SD, tau + 1)
                    k0 = (R * tau) % NSLOT
                    sync.dma_start(
                        LF[8:128, (tau + SKEW) % NLF, 1 : R + 1],
                        BB[0:120, k0 : k0 + R, W : W + 1],
                    ).then_inc(SM, 16)
                sync.wait_ge(SD, T + 1)
                for _ in range(10):
                    sync.wait_ge(SD, T + 1)  # settle spin ~0.5us
                sync.dma_start(
                    out[:, :], BB[120:128, (S_TOTAL - 1) % NSLOT, W : W + 1]
                ).then_inc(dma_out, 16)
            sync.wait_ge(dma_out, 16)

        @block.scalar
        def _(scalar):
            scalar.wait_ge(v_memset, 1)

            def dma_gate(tau):
                need = min(2 + (R * tau + R - 1) // (SKEW * R), N_IN_DMAS)
                scalar.wait_ge(dma_in, 16 * need)

            def act_batch(tau):
                for r in range(R):
                    s = R * tau + r
                    ins = nc.scalar.activation(
                        DTS[:, s % ND, 1 : W + 1],
                        Y[:, :],
                        AF.Abs,
                        bias=XS[:, s : s + 1],
                        scale=-1.0,
                    )
                    if r == R - 1:
                        ins.then_inc(SA, 1)

            for tau in range(3):
                dma_gate(tau)
                act_batch(tau)
            for tau in range(3, T):
                dma_gate(tau)
                scalar.wait_ge(SD, tau - 2)
                act_batch(tau)

        @block.vector
        def _(vector):
            # acts need only XS pads + Y + per-chunk x slices; fire the
            # input-DMA gate right after the XS memset. The remaining
            # memsets complete before the DVE main loop (same queue) and
            # before any cross-engine reader (boundary DMA waits SD>=1).
            nc.vector.memset(XS[:, :], BIG).then_inc(v_memset, 1)
            nc.vector.memset(BB[:, :, :], BIG)
            nc.vector.memset(LF[:, :, :], BIG)
            nc.vector.memset(ZC[:, :], BIG)
            nc.vector.memset(ZC[0:8, :], 0.0)
            nc.vector.memset(DTS[:, :, 0:1], 0.0)
            nc.vector.memset(PTS[:, :, 0:1], BIG)

            for tau in range(T):
                vector.wait_ge(SA, tau + 1)
                lf = LF[:, tau % NLF, :]
                for r in range(R):
                    s = R * tau + r
                    b_prev = BB[:, (s - 1) % NSLOT, :]
                    b_cur = BB[:, s % NSLOT, :]
                    d = DTS[:, s % ND, :]
                    q = QQ[:, s % 2, :]
                    p = PTS[:, s % 3, :]
                    nc.vector.tensor_tensor(
                        q[:, :], b_prev[:, 0:W], d[:, 1 : W + 1], op=ALU.add
                    )
                    nc.vector.tensor_tensor(
                        p[:, 1 : W + 1], q[:, :], b_prev[:, 1 : W + 1], op=ALU.min
                    )
                    if r == 0 and tau >= SKEW:
                        vector.wait_ge(SM, 16 * min(tau - SKEW + 2, T - SKEW))
                    if s == 0:
                        nc.vector.tensor_tensor_scan(
                            b_cur[:, 1 : W + 1],
                            p[:, 1 : W + 1],
                            d[:, 1 : W + 1],
                            ZC[:, 0:1],
                            op0=ALU.min,
                            op1=ALU.add,
                        )
                        ins = nc.vector.memset(b_cur[:, 0:1], BIG)
                    else:
                        ins = nc.vector.tensor_tensor_scan(
                            b_cur[:, 0 : W + 1],
                            p[:, 0 : W + 1],
                            d[:, 0 : W + 1],
                            lf[:, r + 1 : r + 2],
                            op0=ALU.min,
                            op1=ALU.add,
                        )
                    if r == R - 1:
                        ins.then_inc(SD, 1)

            # drain padding after the final scan: give its SBUF writes
            # time to land before the output DMA reads them
            nc.vector.memset(QQ[:, 0, :], 0.0)
            nc.vector.memset(QQ[:, 1, :], 0.0)
            nc.vector.memset(QQ[:, 0, :], 0.0).then_inc(SD, 1)

    return nc


LAST = {}


def kernel(x: np.ndarray, x_target: np.ndarray) -> np.ndarray:
    import os

    x = np.ascontiguousarray(np.asarray(x, np.float32))
    y = np.ascontiguousarray(np.asarray(x_target, np.float32))
    if "nc" not in _CACHE:
        _CACHE["nc"] = _build()
    nc = _CACHE["nc"]
    in_maps = [
        {"x8": x[8 * k : 8 * k + 8], "y8": y[8 * k : 8 * k + 8]}
        for k in range(NCORES)
    ]
    trace = bool(os.environ.get("DTW_TRACE"))
    r = run_bass_kernel_spmd(nc, in_maps, list(range(NCORES)), trace=trace)
    LAST["exec_time_ns"] = r.exec_time_ns
    LAST["profile_json"] = r.profile_json
    LAST["trace_path"] = (
        r.instructions_and_trace[1] if r.instructions_and_trace else None
    )
    res = r.results
    dists = np.concatenate([rr["dists"][:, 0] for rr in res]).astype(np.float32)
    dists = dists / np.float32(2.0 * N)
    return np.float32(np.mean(dists))
